# revision 14
# baseline (speedup 1.0000x reference)
"""Trainium2 Bass kernel for the SelfAttentionBlock problem (8 NeuronCores).

Sharding: MLP data-parallel over rows (each core owns 512 rows),
AllToAll per tensor to switch to head-parallel attention (core c
computes head c for both batches), host concat of outputs.

v3 rewrite (baseline v2 = 207us measured). Trace-driven changes:
  * recv-side loads (qT2/kT2) issued right after their collective's
    trigger instead of after ALL sends -> exp stream starts right
    after the v-L1 gelus instead of ~25us later.
  * v-L2's output gelu computed on DVE via the tanh decomposition
    (tanh shares the ACT table set with exp) so the ACT queue does a
    single gelu->exp table switch and the exp stream is never
    interrupted by a ~2.7us ACT_TABLE_LOAD.
  * v AllToAll split by batch (v-L2 row-chunks mo0,1 == batch 0) so
    PV(b=0) starts ~10us earlier; PV/norms chase the exp stream.
  * scores matmuls interleaved with v-L2 chunks on the PE queue.
  * L1 gelus batched in [128,1024] pairs (fewer ACT calls).
  * startup DMA order: xt + first w1q chunk first -> first matmul
    several us earlier.
"""

import math

import ml_dtypes
import numpy as np

import concourse.bass as bass
import concourse.mybir as mybir
from concourse import bacc, tile
from concourse.bass_utils import run_bass_kernel_spmd

N_CORES = 8
N, S, D, H = 2, 2048, 512, 2048
HEADS = 8
Dh = D // HEADS            # 64
RPC = S // N_CORES         # 256 rows per core per batch
ROWS = N * RPC             # 512 rows per core
KT1 = D // 128             # 4 layer-1 contraction tiles
MT1 = H // 128             # 16 layer-1 out tiles == layer-2 contraction tiles
MT2 = D // 128             # 4 layer-2 out tiles
QC = S // 512              # 4 q-chunks per batch

F32 = mybir.dt.float32
BF16 = mybir.dt.bfloat16
AF = mybir.ActivationFunctionType
ALU = mybir.AluOpType

GC = math.sqrt(2.0 / math.pi)   # tanh-gelu scale
GC1 = 0.044715

# const blob column offsets
CF_B1 = {"k": 0, "q": 16, "v": 32}
CF_B2 = {"k": 48, "q": 52}
CF_W = 120
CB_BAND = 0                # [128, 1024] causal band
CB_B2V = 1024              # row 0: v second-layer bias [512]
CB_W = 1536


def _build():
    nc = bacc.Bacc("TRN2", target_bir_lowering=False, debug=False,
                   num_devices=N_CORES)

    xt_d = nc.dram_tensor("xt", [128, KT1 * ROWS], BF16, kind="ExternalInput")
    w1_d = {t: nc.dram_tensor(f"w1{t}", [128, MT1 * KT1 * 128], BF16,
                              kind="ExternalInput") for t in "kqv"}
    w2_d = {t: nc.dram_tensor(f"w2{t}", [128, MT1 * D], BF16,
                              kind="ExternalInput") for t in "kqv"}
    cf32_d = nc.dram_tensor("cf32", [128, CF_W], F32, kind="ExternalInput")
    cbf_d = nc.dram_tensor("cbf", [128, CB_W], BF16, kind="ExternalInput")

    kT_out = nc.dram_tensor("kT_out", [HEADS, Dh, ROWS], BF16,
                            kind="ExternalOutput")
    v_out = nc.dram_tensor("v_out", [128, HEADS * MT2 * Dh], BF16,
                           kind="ExternalOutput")
    aT_out = nc.dram_tensor("attn_outT", [N, Dh, S], BF16,
                            kind="ExternalOutput")

    with tile.TileContext(nc) as tc:
        with (
            tc.tile_pool(name="dram", bufs=1, space="DRAM") as dp,
            tc.tile_pool(name="cst", bufs=1) as cst,
            tc.tile_pool(name="w1p", bufs=2) as w1p,
            tc.tile_pool(name="w2p", bufs=2) as w2p,
            tc.tile_pool(name="h1p", bufs=9) as h1p,
            tc.tile_pool(name="l2p", bufs=2) as l2p,
            tc.tile_pool(name="gel", bufs=4) as gelp,
            tc.tile_pool(name="exp", bufs=34) as expp,
            tc.tile_pool(name="sm", bufs=2) as smp,
            tc.tile_pool(name="ps", bufs=2, space="PSUM") as psp,
            tc.tile_pool(name="sc", bufs=2, space="PSUM") as scp,
        ):
            # q and k share ONE AllToAll (collectives serialize on the CC
            # core at ~9us each after a ~70us first-mesh setup, so fewer +
            # later-triggered collectives win)
            send_qk = dp.tile([HEADS, Dh, 2, ROWS], BF16, tag="send_qk",
                              name="send_qk")
            recv_qk = dp.tile([HEADS, Dh, 2, ROWS], BF16, tag="recv_qk",
                              name="recv_qk")
            send_v = dp.tile([HEADS, 128, MT2, Dh], BF16, tag="send_v",
                             name="send_v")
            recv_v = dp.tile([HEADS, 128, MT2, Dh], BF16, tag="recv_v",
                             name="recv_v")
            warm_s = dp.tile([HEADS, 64], BF16, tag="warm_s", name="warm_s")
            warm_r = dp.tile([HEADS, 64], BF16, tag="warm_r", name="warm_r")

            # warm up collectives firmware + cross-core sync
            nc.gpsimd.collective_compute(
                "AllToAll", mybir.AluOpType.bypass,
                replica_groups=[list(range(N_CORES))],
                ins=[warm_s.opt()], outs=[warm_r.opt()])

            # ---- persistent SBUF tiles / early DMAs (Sync queue) ----
            xt = cst.tile([128, KT1 * ROWS], BF16, tag="xt")
            nc.sync.dma_start(xt[:], xt_d[:])
            w1sb = {"q": w1p.tile([128, MT1 * KT1 * 128], BF16, tag="w1",
                                  name="w1_q")}
            # first 2 m-tiles worth so L1-q m0 can start immediately
            nc.sync.dma_start(w1sb["q"][:, 0:1024], w1_d["q"][:, 0:1024])
            cf32 = cst.tile([128, CF_W], F32, tag="cf32")
            nc.sync.dma_start(cf32[:], cf32_d[:])
            nc.sync.dma_start(w1sb["q"][:, 1024:4096], w1_d["q"][:, 1024:4096])
            nc.sync.dma_start(w1sb["q"][:, 4096:8192], w1_d["q"][:, 4096:8192])
            w2sb = {"q": w2p.tile([128, MT1 * D], BF16, tag="w2", name="w2_q")}
            nc.sync.dma_start(w2sb["q"][:], w2_d["q"][:])
            cbf = cst.tile([128, CB_W], BF16, tag="cbf")
            nc.sync.dma_start(cbf[:], cbf_d[:])

            ones_row = cst.tile([1, 128], BF16, tag="ones_row")
            nc.vector.memset(ones_row[:], 1.0)
            kT2 = cst.tile([128, HEADS * RPC], BF16, tag="kT2")
            qT2 = cst.tile([128, HEADS * RPC], BF16, tag="qT2")
            vaug = cst.tile([128, HEADS * N * 2 * 65], BF16, tag="vaug")
            with nc.allow_low_precision(reason="ones fill"):
                nc.vector.memset(
                    vaug[:].rearrange("p (g e) -> p g e", e=65)[:, :, 64:65],
                    1.0)
            oT_all = cst.tile([Dh, N * S], BF16, tag="oT")

            def load_w(t):
                w1sb[t] = w1p.tile([128, MT1 * KT1 * 128], BF16, tag="w1",
                                   name=f"w1_{t}")
                nc.sync.dma_start(w1sb[t][:], w1_d[t][:])
                w2sb[t] = w2p.tile([128, MT1 * D], BF16, tag="w2",
                                   name=f"w2_{t}")
                nc.sync.dma_start(w2sb[t][:], w2_d[t][:])

            def l1(t, after_pair=None):
                """Layer 1: 8 pair-psum tiles, gelu in [128,1024] batches."""
                h1_t = []
                for mp in range(MT1 // 2):
                    pp = psp.tile([128, 1024], F32, tag="ps",
                                  name=f"p1{t}{mp}")
                    for half in range(2):
                        m = 2 * mp + half
                        for kt in range(KT1):
                            nc.tensor.matmul(
                                pp[:, half * 512:(half + 1) * 512],
                                w1sb[t][:, m * 512 + kt * 128:
                                        m * 512 + (kt + 1) * 128],
                                xt[:, kt * ROWS:(kt + 1) * ROWS],
                                start=(kt == 0), stop=(kt == KT1 - 1))
                    h1 = h1p.tile([128, 1024], BF16, tag="h1",
                                  name=f"h1{t}{mp}")
                    # bias is per-partition scalar; pair shares none -> use
                    # two separate bias columns via two activations? No:
                    # bias differs between the two halves, so do 2 calls
                    # only when biases differ. Biases are zero in this
                    # problem's setup but keep correctness: use bias AP per
                    # half.
                    nc.scalar.activation(
                        h1[:, 0:512], pp[:, 0:512], AF.Gelu_apprx_tanh,
                        bias=cf32[:, CF_B1[t] + 2 * mp:CF_B1[t] + 2 * mp + 1])
                    nc.scalar.activation(
                        h1[:, 512:1024], pp[:, 512:1024], AF.Gelu_apprx_tanh,
                        bias=cf32[:, CF_B1[t] + 2 * mp + 1:
                                  CF_B1[t] + 2 * mp + 2])
                    h1_t.append(h1)
                    if after_pair:
                        after_pair(mp)
                return h1_t

            def l2_qk(t, h1_t):
                """Layer 2 for q/k: out transposed [D, ROWS], bias on DVE."""
                l2 = l2p.tile([128, MT2 * 512], BF16, tag="l2", name=f"l2{t}")
                for mo in range(MT2):
                    pp = psp.tile([128, 512], F32, tag="ps", name=f"p2{t}{mo}")
                    for kt in range(MT1):
                        nc.tensor.matmul(
                            pp[:],
                            w2sb[t][:, kt * D + mo * 128:
                                    kt * D + (mo + 1) * 128],
                            h1_t[kt // 2][:, (kt % 2) * 512:
                                          (kt % 2) * 512 + 512],
                            start=(kt == 0), stop=(kt == MT1 - 1))
                    with nc.allow_low_precision(reason="bf16 out"):
                        nc.vector.tensor_scalar_add(
                            l2[:, mo * 512:(mo + 1) * 512], pp[:],
                            cf32[:, CF_B2[t] + mo:CF_B2[t] + mo + 1])
                return l2

            # ---------------- MLP q ----------------
            h1q = l1("q", after_pair=lambda mp: load_w("k") if mp == 0
                     else None)
            l2q = l2_qk("q", h1q)
            nc.sync.dma_start(
                send_qk[:, :, 0, :]
                .rearrange("(m m2) p r -> (m2 p) m r", m2=2),
                l2q[:].rearrange("p (m r) -> p m r", m=MT2))
            load_w("v")

            # ---------------- MLP k ----------------
            h1k = l1("k")
            l2k = l2_qk("k", h1k)
            nc.sync.dma_start(
                send_qk[:, :, 1, :]
                .rearrange("(m m2) p r -> (m2 p) m r", m2=2),
                l2k[:].rearrange("p (m r) -> p m r", m=MT2))
            nc.gpsimd.collective_compute(
                "AllToAll", mybir.AluOpType.bypass,
                replica_groups=[list(range(N_CORES))],
                ins=[send_qk.opt()], outs=[recv_qk.opt()])
            nc.sync.dma_start(
                kT_out.ap().rearrange("(m m2) p r -> (m2 p) m r", m2=2),
                l2k[:].rearrange("p (m r) -> p m r", m=MT2))
            # recv q/k on the Act HWDGE queue: can't block Sync-queue sends
            for b in range(N):
                nc.scalar.dma_start(
                    qT2[b * 64:(b + 1) * 64, :]
                    .rearrange("p (j r) -> p j r", j=HEADS),
                    recv_qk[:, :, 0, b * RPC:(b + 1) * RPC]
                    .rearrange("j p r -> p j r"))
                nc.scalar.dma_start(
                    kT2[b * 64:(b + 1) * 64, :]
                    .rearrange("p (j r) -> p j r", j=HEADS),
                    recv_qk[:, :, 1, b * RPC:(b + 1) * RPC]
                    .rearrange("j p r -> p j r"))

            # ---------------- MLP v layer 1 ----------------
            h1v = l1("v")

            # ---------------- attention helpers ----------------
            exps = {}

            def score_group(qc, g):
                """One group: 2 k-tiles x 2 batches, paired matmuls + exp."""
                if g == 0:
                    exps[(0, qc)] = []
                    exps[(1, qc)] = []
                pps = [scp.tile([128, 1024], F32, tag="sc",
                                name=f"sc{b}{qc}{g}") for b in range(N)]
                for h in range(2):
                    kt = 2 * g + h
                    for b in range(N):
                        nc.tensor.matmul(
                            pps[b][:, h * 512:(h + 1) * 512],
                            kT2[b * 64:(b + 1) * 64,
                                kt * 128:(kt + 1) * 128],
                            qT2[b * 64:(b + 1) * 64,
                                qc * 512:(qc + 1) * 512],
                            start=True, stop=True)
                for b in range(N):
                    ex = expp.tile([128, 1024], BF16, tag="exp",
                                   name=f"ex{b}{qc}{g}")
                    nc.scalar.activation(ex[:], pps[b][:], AF.Exp,
                                         scale=0.125)
                    for h in range(2):
                        kt = 2 * g + h
                        o = kt * 128 - qc * 512
                        if o >= 0:   # diagonal tile: causal band mask
                            with nc.allow_low_precision(reason="mask"):
                                nc.vector.tensor_mul(
                                    ex[:, h * 512:(h + 1) * 512],
                                    ex[:, h * 512:(h + 1) * 512],
                                    cbf[:, CB_BAND + 512 - o:
                                        CB_BAND + 1024 - o])
                    exps[(b, qc)].append(ex)

            l2v = l2p.tile([128, HEADS * MT2 * Dh], BF16, tag="l2",
                           name="l2v")

            def v_l2_chunk(mo):
                """v layer-2 row chunk mo: matmul + bias + DVE tanh-gelu."""
                pp = psp.tile([128, 512], F32, tag="ps", name=f"p2v{mo}")
                for kt in range(MT1):
                    nc.tensor.matmul(
                        pp[:],
                        h1v[kt // 2][:, (kt % 2) * 512 + mo * 128:
                                     (kt % 2) * 512 + (mo + 1) * 128],
                        w2sb["v"][:, kt * D:(kt + 1) * D],
                        start=(kt == 0), stop=False)
                nc.tensor.matmul(pp[:], ones_row[:],
                                 cbf[0:1, CB_B2V:CB_B2V + 512],
                                 start=False, stop=True)
                # gelu on DVE+tanh (tanh shares the exp ACT table set):
                # y = 0.5*x*(1+tanh(GC*(x+GC1*x^3)))
                x = gelp.tile([128, 512], BF16, tag="gx", name=f"gx{mo}")
                with nc.allow_low_precision(reason="bf16 gelu"):
                    nc.vector.tensor_copy(x[:], pp[:])
                    u = gelp.tile([128, 512], BF16, tag="gtmp",
                                  name=f"gu{mo}")
                    nc.vector.tensor_mul(u[:], x[:], x[:])          # x^2
                    t1 = gelp.tile([128, 512], BF16, tag="gtmp",
                                   name=f"gt1{mo}")
                    # t1 = (x*GC*GC1)*u = GC*GC1*x^3
                    nc.vector.scalar_tensor_tensor(
                        t1[:], x[:], GC * GC1, u[:], ALU.mult, ALU.mult)
                    w = gelp.tile([128, 512], BF16, tag="gtmp",
                                  name=f"gw{mo}")
                    # w = (x*GC) + t1
                    nc.vector.scalar_tensor_tensor(
                        w[:], x[:], GC, t1[:], ALU.mult, ALU.add)
                    t = gelp.tile([128, 512], BF16, tag="gtmp",
                                  name=f"gt{mo}")
                    nc.scalar.activation(t[:], w[:], AF.Tanh)
                    # t2 = t*0.5 + 0.5
                    t2 = gelp.tile([128, 512], BF16, tag="gtmp",
                                   name=f"gt2{mo}")
                    nc.vector.tensor_scalar(t2[:], t[:], 0.5, 0.5,
                                            ALU.mult, ALU.add)
                    # out = (x*1.0) * t2 -> l2v slice, laid out (c, mo, d)
                    nc.vector.scalar_tensor_tensor(
                        l2v[:].rearrange("p (c m d) -> p c m d", c=HEADS,
                                         m=MT2)[:, :, mo, :],
                        x[:], 1.0, t2[:], ALU.mult, ALU.mult)

            def send_v_all():
                nc.sync.dma_start(
                    send_v[:].rearrange("c p m d -> p c m d"),
                    l2v[:].rearrange("p (c m d) -> p c m d", c=HEADS,
                                     m=MT2))
                nc.gpsimd.collective_compute(
                    "AllToAll", mybir.AluOpType.bypass,
                    replica_groups=[list(range(N_CORES))],
                    ins=[send_v.opt()], outs=[recv_v.opt()])
                # recv side: vaug loads (m = b*2+h row-chunk)
                for b in range(N):
                    for h in range(2):
                        nc.sync.dma_start(
                            vaug[:].rearrange("p (j b h e) -> p j b h e",
                                              j=HEADS, b=N, h=2)
                            [:, :, b, h, 0:64],
                            recv_v[:, :, b * 2 + h, :]
                            .rearrange("j p d -> p j d"))

            po = {}

            def pv(b, qc):
                nk = 4 * qc + 4
                p = psp.tile([128, 1024], F32, tag="ps", name=f"po{b}{qc}")
                po[(b, qc)] = p
                for kt in range(nk):
                    g, h = kt // 2, kt % 2
                    gidx = (kt // 2) * 4 + b * 2 + (kt % 2)
                    nc.tensor.matmul(
                        p[0:65, 0:512],
                        vaug[:, gidx * 65:(gidx + 1) * 65],
                        exps[(b, qc)][g][:, h * 512:(h + 1) * 512],
                        start=(kt == 0), stop=(kt == nk - 1))

            def norm(b, qc):
                p = po[(b, qc)]
                d_sb = smp.tile([1, 512], F32, tag="d", name=f"d{b}{qc}")
                nc.vector.tensor_copy(d_sb[:], p[64:65, 0:512])
                r = smp.tile([1, 512], F32, tag="r", name=f"r{b}{qc}")
                with nc.allow_low_precision(reason="approx recip"):
                    nc.vector.reciprocal_approx_fast(r[:], d_sb[:])
                rb = smp.tile([64, 512], F32, tag="rb", name=f"rb{b}{qc}")
                nc.gpsimd.partition_broadcast(rb[:], r[:], channels=64)
                with nc.allow_low_precision(reason="bf16"):
                    nc.vector.tensor_mul(
                        oT_all[:, b * S + qc * 512:b * S + (qc + 1) * 512],
                        p[0:64, 0:512], rb[:])
                nc.sync.dma_start(
                    aT_out[b, :, qc * 512:(qc + 1) * 512],
                    oT_all[:, b * S + qc * 512:b * S + (qc + 1) * 512])

            # ------- interleaved scores + v-L2 + sends (PE queue order) ----
            groups = [(qc, g) for qc in (3, 2, 1, 0)
                      for g in range(2 * (qc + 1))]   # 20 groups
            # PE interleave: 2 groups, then a v-L2 chunk, ...
            gi = 0

            def emit_groups(n):
                nonlocal gi
                for _ in range(n):
                    if gi < len(groups):
                        score_group(*groups[gi])
                        gi += 1

            emit_groups(2)
            v_l2_chunk(0)
            emit_groups(2)
            v_l2_chunk(1)
            emit_groups(2)
            v_l2_chunk(2)
            emit_groups(2)
            v_l2_chunk(3)
            send_v_all()
            nc.sync.dma_start(v_out.ap(), l2v[:])
            emit_groups(len(groups))

            # ---------------- PV + norms ----------------
            pv(0, 3)
            pv(1, 3)
            norm(0, 3)
            pv(0, 2)
            norm(1, 3)
            pv(1, 2)
            norm(0, 2)
            pv(0, 1)
            norm(1, 2)
            pv(1, 1)
            norm(0, 1)
            pv(0, 0)
            norm(1, 1)
            pv(1, 0)
            norm(0, 0)
            norm(1, 0)

    nc.compile()
    return nc


_COMPILED = None


def _get_compiled():
    global _COMPILED
    if _COMPILED is None:
        _COMPILED = _build()
    return _COMPILED


def _band_mask():
    return (np.arange(1024, dtype=np.int32)[None, :]
            >= (np.arange(128, dtype=np.int32)[:, None] + 512)).astype(
                np.float32)


def _bf16(a):
    return np.ascontiguousarray(np.asarray(a, dtype=np.float32)
                                .astype(ml_dtypes.bfloat16))


def _pack_w1(w):            # [512, 2048] -> [128, (m kt 128)]
    w = np.asarray(w, np.float32)
    return _bf16(w.reshape(KT1, 128, MT1, 128).transpose(1, 2, 0, 3)
                 .reshape(128, MT1 * KT1 * 128))


def _pack_w2(w):            # [2048, 512] -> [128, (kt d)]
    w = np.asarray(w, np.float32)
    return _bf16(w.reshape(MT1, 128, D).transpose(1, 0, 2)
                 .reshape(128, MT1 * D))


def _make_in_maps(x, qW1, qb1, qW2, qb2, kW1, kb1, kW2, kb2, vW1, vb1,
                  vW2, vb2):
    x = np.asarray(x, np.float32)
    cf32 = np.zeros((128, CF_W), np.float32)
    for t, b1 in (("k", kb1), ("q", qb1), ("v", vb1)):
        cf32[:, CF_B1[t]:CF_B1[t] + 16] = np.asarray(b1, np.float32) \
            .reshape(16, 128).T
    for t, b2 in (("k", kb2), ("q", qb2)):
        cf32[:, CF_B2[t]:CF_B2[t] + 4] = np.asarray(b2, np.float32) \
            .reshape(4, 128).T
    cbf = np.zeros((128, CB_W), np.float32)
    cbf[:, CB_BAND:CB_BAND + 1024] = _band_mask()
    cbf[0, CB_B2V:CB_B2V + 512] = np.asarray(vb2, np.float32)
    shared = {
        "w1q": _pack_w1(qW1), "w1k": _pack_w1(kW1), "w1v": _pack_w1(vW1),
        "w2q": _pack_w2(qW2), "w2k": _pack_w2(kW2), "w2v": _pack_w2(vW2),
        "cf32": cf32, "cbf": cbf.astype(ml_dtypes.bfloat16),
    }
    in_maps = []
    for c in range(N_CORES):
        xc = np.concatenate([x[b, c * RPC:(c + 1) * RPC, :]
                             for b in range(N)], 0)       # [ROWS, D]
        xT = np.ascontiguousarray(xc.T)                   # [D, ROWS]
        im = dict(shared)
        im["xt"] = _bf16(xT.reshape(KT1, 128, ROWS).transpose(1, 0, 2)
                         .reshape(128, KT1 * ROWS))
        in_maps.append(im)
    return in_maps


def _assemble(res):
    k_full = np.empty((N, S, D), np.float32)
    v_full = np.empty((N, S, D), np.float32)
    out_full = np.empty((N, S, D), np.float32)
    for j in range(N_CORES):
        kT_j = np.asarray(res[j]["kT_out"], np.float32)   # [8, 64, ROWS]
        v_j = np.asarray(res[j]["v_out"], np.float32) \
            .reshape(128, HEADS, N, 2, Dh).transpose(2, 3, 0, 1, 4) \
            .reshape(N, RPC, D)                           # [N, RPC, D]
        aT_j = np.asarray(res[j]["attn_outT"], np.float32)  # [N, Dh, S]
        kk = kT_j.reshape(HEADS, Dh, N, RPC).transpose(2, 3, 0, 1) \
            .reshape(N, RPC, D)
        for b in range(N):
            k_full[b, j * RPC:(j + 1) * RPC, :] = kk[b]
            v_full[b, j * RPC:(j + 1) * RPC, :] = v_j[b]
            out_full[b, :, j * Dh:(j + 1) * Dh] = aT_j[b].T
    return k_full, v_full, out_full


def kernel(**inputs):
    nc = _get_compiled()
    in_maps = _make_in_maps(**inputs)
    res = run_bass_kernel_spmd(nc, in_maps, list(range(N_CORES))).results
    return _assemble(res)


# revision 17
# speedup vs baseline: 1.0684x; 1.0684x over previous
"""Trainium2 Bass kernel for the SelfAttentionBlock problem (8 NeuronCores).

Sharding: MLP data-parallel over rows (each core owns 512 rows),
AllToAll per tensor to switch to head-parallel attention (core c
computes head c for both batches), host concat of outputs.

v3 rewrite (baseline v2 = 207us measured). Trace-driven changes:
  * recv-side loads (qT2/kT2) issued right after their collective's
    trigger instead of after ALL sends -> exp stream starts right
    after the v-L1 gelus instead of ~25us later.
  * v-L2's output gelu computed on DVE via the tanh decomposition
    (tanh shares the ACT table set with exp) so the ACT queue does a
    single gelu->exp table switch and the exp stream is never
    interrupted by a ~2.7us ACT_TABLE_LOAD.
  * v AllToAll split by batch (v-L2 row-chunks mo0,1 == batch 0) so
    PV(b=0) starts ~10us earlier; PV/norms chase the exp stream.
  * scores matmuls interleaved with v-L2 chunks on the PE queue.
  * L1 gelus batched in [128,1024] pairs (fewer ACT calls).
  * startup DMA order: xt + first w1q chunk first -> first matmul
    several us earlier.
"""

import math

import ml_dtypes
import numpy as np

import concourse.bass as bass
import concourse.mybir as mybir
from concourse import bacc, tile
from concourse.bass_utils import run_bass_kernel_spmd

N_CORES = 8
N, S, D, H = 2, 2048, 512, 2048
HEADS = 8
Dh = D // HEADS            # 64
RPC = S // N_CORES         # 256 rows per core per batch
ROWS = N * RPC             # 512 rows per core
KT1 = D // 128             # 4 layer-1 contraction tiles
MT1 = H // 128             # 16 layer-1 out tiles == layer-2 contraction tiles
MT2 = D // 128             # 4 layer-2 out tiles
QC = S // 512              # 4 q-chunks per batch

F32 = mybir.dt.float32
BF16 = mybir.dt.bfloat16
AF = mybir.ActivationFunctionType
ALU = mybir.AluOpType

GC = math.sqrt(2.0 / math.pi)   # tanh-gelu scale
GC1 = 0.044715

# const blob column offsets
CF_B1 = {"k": 0, "q": 16, "v": 32}
CF_B2 = {"k": 48, "q": 52}
CF_W = 120
CB_BAND = 0                # [128, 1024] causal band
CB_B2V = 1024              # row 0: v second-layer bias [512]
CB_W = 1536


def _build():
    nc = bacc.Bacc("TRN2", target_bir_lowering=False, debug=False,
                   num_devices=N_CORES)

    xt_d = nc.dram_tensor("xt", [128, KT1 * ROWS], BF16, kind="ExternalInput")
    w1_d = {t: nc.dram_tensor(f"w1{t}", [128, MT1 * KT1 * 128], BF16,
                              kind="ExternalInput") for t in "kqv"}
    w2_d = {t: nc.dram_tensor(f"w2{t}", [128, MT1 * D], BF16,
                              kind="ExternalInput") for t in "kqv"}
    cf32_d = nc.dram_tensor("cf32", [128, CF_W], F32, kind="ExternalInput")
    cbf_d = nc.dram_tensor("cbf", [128, CB_W], BF16, kind="ExternalInput")

    kT_out = nc.dram_tensor("kT_out", [HEADS, Dh, ROWS], BF16,
                            kind="ExternalOutput")
    v_out = nc.dram_tensor("v_out", [128, HEADS * MT2 * Dh], BF16,
                           kind="ExternalOutput")
    aT_out = nc.dram_tensor("attn_outT", [N, Dh, S], BF16,
                            kind="ExternalOutput")

    with tile.TileContext(nc) as tc:
        with (
            tc.tile_pool(name="dram", bufs=1, space="DRAM") as dp,
            tc.tile_pool(name="cst", bufs=1) as cst,
            tc.tile_pool(name="w1p", bufs=2) as w1p,
            tc.tile_pool(name="w2p", bufs=2) as w2p,
            tc.tile_pool(name="h1p", bufs=9) as h1p,
            tc.tile_pool(name="l2p", bufs=2) as l2p,
            tc.tile_pool(name="gel", bufs=4) as gelp,
            tc.tile_pool(name="exp", bufs=34) as expp,
            tc.tile_pool(name="sm", bufs=2) as smp,
            tc.tile_pool(name="ps", bufs=2, space="PSUM") as psp,
            tc.tile_pool(name="sc", bufs=2, space="PSUM") as scp,
        ):
            # q and k share ONE AllToAll (collectives serialize on the CC
            # core at ~9us each after a ~70us first-mesh setup, so fewer +
            # later-triggered collectives win)
            send_qk = dp.tile([HEADS, Dh, 2, ROWS], BF16, tag="send_qk",
                              name="send_qk")
            recv_qk = dp.tile([HEADS, Dh, 2, ROWS], BF16, tag="recv_qk",
                              name="recv_qk")
            send_v = dp.tile([HEADS, 128, MT2, Dh], BF16, tag="send_v",
                             name="send_v")
            recv_v = dp.tile([HEADS, 128, MT2, Dh], BF16, tag="recv_v",
                             name="recv_v")
            warm_s = dp.tile([HEADS, 64], BF16, tag="warm_s", name="warm_s")
            warm_r = dp.tile([HEADS, 64], BF16, tag="warm_r", name="warm_r")

            # warm up collectives firmware + cross-core sync
            nc.gpsimd.collective_compute(
                "AllToAll", mybir.AluOpType.bypass,
                replica_groups=[list(range(N_CORES))],
                ins=[warm_s.opt()], outs=[warm_r.opt()])

            # ---- persistent SBUF tiles / early DMAs (Sync queue) ----
            xt = cst.tile([128, KT1 * ROWS], BF16, tag="xt")
            nc.sync.dma_start(xt[:], xt_d[:])
            w1sb = {"q": w1p.tile([128, MT1 * KT1 * 128], BF16, tag="w1",
                                  name="w1_q")}
            # first 2 m-tiles worth so L1-q m0 can start immediately
            nc.sync.dma_start(w1sb["q"][:, 0:1024], w1_d["q"][:, 0:1024])
            cf32 = cst.tile([128, CF_W], F32, tag="cf32")
            nc.sync.dma_start(cf32[:], cf32_d[:])
            nc.sync.dma_start(w1sb["q"][:, 1024:4096], w1_d["q"][:, 1024:4096])
            nc.sync.dma_start(w1sb["q"][:, 4096:8192], w1_d["q"][:, 4096:8192])
            w2sb = {"q": w2p.tile([128, MT1 * D], BF16, tag="w2", name="w2_q")}
            nc.sync.dma_start(w2sb["q"][:], w2_d["q"][:])
            cbf = cst.tile([128, CB_W], BF16, tag="cbf")
            nc.sync.dma_start(cbf[:], cbf_d[:])

            ones_row = cst.tile([1, 128], BF16, tag="ones_row")
            nc.vector.memset(ones_row[:], 1.0)
            kT2 = cst.tile([128, HEADS * RPC], BF16, tag="kT2")
            qT2 = cst.tile([128, HEADS * RPC], BF16, tag="qT2")
            vaug = cst.tile([128, HEADS * N * 2 * 65], BF16, tag="vaug")
            with nc.allow_low_precision(reason="ones fill"):
                nc.vector.memset(
                    vaug[:].rearrange("p (g e) -> p g e", e=65)[:, :, 64:65],
                    1.0)
            oT_all = cst.tile([Dh, N * S], BF16, tag="oT")

            def load_w(t):
                w1sb[t] = w1p.tile([128, MT1 * KT1 * 128], BF16, tag="w1",
                                   name=f"w1_{t}")
                nc.sync.dma_start(w1sb[t][:], w1_d[t][:])
                w2sb[t] = w2p.tile([128, MT1 * D], BF16, tag="w2",
                                   name=f"w2_{t}")
                nc.sync.dma_start(w2sb[t][:], w2_d[t][:])

            def l1(t, after_pair=None):
                """Layer 1: 8 pair-psum tiles, gelu in [128,1024] batches."""
                h1_t = []
                for mp in range(MT1 // 2):
                    pp = psp.tile([128, 1024], F32, tag="ps",
                                  name=f"p1{t}{mp}")
                    for half in range(2):
                        m = 2 * mp + half
                        for kt in range(KT1):
                            nc.tensor.matmul(
                                pp[:, half * 512:(half + 1) * 512],
                                w1sb[t][:, m * 512 + kt * 128:
                                        m * 512 + (kt + 1) * 128],
                                xt[:, kt * ROWS:(kt + 1) * ROWS],
                                start=(kt == 0), stop=(kt == KT1 - 1))
                    h1 = h1p.tile([128, 1024], BF16, tag="h1",
                                  name=f"h1{t}{mp}")
                    # bias is per-partition scalar; pair shares none -> use
                    # two separate bias columns via two activations? No:
                    # bias differs between the two halves, so do 2 calls
                    # only when biases differ. Biases are zero in this
                    # problem's setup but keep correctness: use bias AP per
                    # half.
                    nc.scalar.activation(
                        h1[:, 0:512], pp[:, 0:512], AF.Gelu_apprx_tanh,
                        bias=cf32[:, CF_B1[t] + 2 * mp:CF_B1[t] + 2 * mp + 1])
                    nc.scalar.activation(
                        h1[:, 512:1024], pp[:, 512:1024], AF.Gelu_apprx_tanh,
                        bias=cf32[:, CF_B1[t] + 2 * mp + 1:
                                  CF_B1[t] + 2 * mp + 2])
                    h1_t.append(h1)
                    if after_pair:
                        after_pair(mp)
                return h1_t

            def l2_qk(t, h1_t):
                """Layer 2 for q/k: out transposed [D, ROWS], bias on DVE."""
                l2 = l2p.tile([128, MT2 * 512], BF16, tag="l2", name=f"l2{t}")
                for mo in range(MT2):
                    pp = psp.tile([128, 512], F32, tag="ps", name=f"p2{t}{mo}")
                    for kt in range(MT1):
                        nc.tensor.matmul(
                            pp[:],
                            w2sb[t][:, kt * D + mo * 128:
                                    kt * D + (mo + 1) * 128],
                            h1_t[kt // 2][:, (kt % 2) * 512:
                                          (kt % 2) * 512 + 512],
                            start=(kt == 0), stop=(kt == MT1 - 1))
                    with nc.allow_low_precision(reason="bf16 out"):
                        nc.vector.tensor_scalar_add(
                            l2[:, mo * 512:(mo + 1) * 512], pp[:],
                            cf32[:, CF_B2[t] + mo:CF_B2[t] + mo + 1])
                return l2

            # ---------------- MLP q ----------------
            h1q = l1("q", after_pair=lambda mp: load_w("k") if mp == 0
                     else None)
            l2q = l2_qk("q", h1q)
            nc.sync.dma_start(
                send_qk[:, :, 0, :]
                .rearrange("(m m2) p r -> (m2 p) m r", m2=2),
                l2q[:].rearrange("p (m r) -> p m r", m=MT2))
            load_w("v")

            # ---------------- MLP k ----------------
            h1k = l1("k")
            l2k = l2_qk("k", h1k)
            nc.sync.dma_start(
                send_qk[:, :, 1, :]
                .rearrange("(m m2) p r -> (m2 p) m r", m2=2),
                l2k[:].rearrange("p (m r) -> p m r", m=MT2))
            nc.gpsimd.collective_compute(
                "AllToAll", mybir.AluOpType.bypass,
                replica_groups=[list(range(N_CORES))],
                ins=[send_qk.opt()], outs=[recv_qk.opt()])
            # recv q/k on the Act HWDGE queue: can't block Sync-queue sends
            for b in range(N):
                nc.scalar.dma_start(
                    qT2[b * 64:(b + 1) * 64, :]
                    .rearrange("p (j r) -> p j r", j=HEADS),
                    recv_qk[:, :, 0, b * RPC:(b + 1) * RPC]
                    .rearrange("j p r -> p j r"))
                nc.scalar.dma_start(
                    kT2[b * 64:(b + 1) * 64, :]
                    .rearrange("p (j r) -> p j r", j=HEADS),
                    recv_qk[:, :, 1, b * RPC:(b + 1) * RPC]
                    .rearrange("j p r -> p j r"))

            # ---------------- MLP v layer 1 ----------------
            h1v = l1("v")

            # ---------------- attention helpers ----------------
            exps = {}

            def score_group(qc, g):
                """One group: 2 k-tiles x 2 batches, paired matmuls + exp."""
                if g == 0:
                    exps[(0, qc)] = []
                    exps[(1, qc)] = []
                pps = [scp.tile([128, 1024], F32, tag="sc",
                                name=f"sc{b}{qc}{g}") for b in range(N)]
                for h in range(2):
                    kt = 2 * g + h
                    for b in range(N):
                        nc.tensor.matmul(
                            pps[b][:, h * 512:(h + 1) * 512],
                            kT2[b * 64:(b + 1) * 64,
                                kt * 128:(kt + 1) * 128],
                            qT2[b * 64:(b + 1) * 64,
                                qc * 512:(qc + 1) * 512],
                            start=True, stop=True)
                for b in range(N):
                    ex = expp.tile([128, 1024], BF16, tag="exp",
                                   name=f"ex{b}{qc}{g}")
                    nc.scalar.activation(ex[:], pps[b][:], AF.Exp,
                                         scale=0.125)
                    for h in range(2):
                        kt = 2 * g + h
                        o = kt * 128 - qc * 512
                        if o >= 0:   # diagonal tile: causal band mask
                            with nc.allow_low_precision(reason="mask"):
                                nc.vector.tensor_mul(
                                    ex[:, h * 512:(h + 1) * 512],
                                    ex[:, h * 512:(h + 1) * 512],
                                    cbf[:, CB_BAND + 512 - o:
                                        CB_BAND + 1024 - o])
                    exps[(b, qc)].append(ex)

            l2v = l2p.tile([128, HEADS * MT2 * Dh], BF16, tag="l2",
                           name="l2v")

            def v_l2_chunk(mo):
                """v layer-2 row chunk mo: matmul + bias + DVE tanh-gelu."""
                pp = psp.tile([128, 512], F32, tag="ps", name=f"p2v{mo}")
                for kt in range(MT1):
                    nc.tensor.matmul(
                        pp[:],
                        h1v[kt // 2][:, (kt % 2) * 512 + mo * 128:
                                     (kt % 2) * 512 + (mo + 1) * 128],
                        w2sb["v"][:, kt * D:(kt + 1) * D],
                        start=(kt == 0), stop=False)
                nc.tensor.matmul(pp[:], ones_row[:],
                                 cbf[0:1, CB_B2V:CB_B2V + 512],
                                 start=False, stop=True)
                # gelu on DVE+tanh (tanh shares the exp ACT table set):
                # y = 0.5*x*(1+tanh(GC*(x+GC1*x^3)))
                x = gelp.tile([128, 512], BF16, tag="gx", name=f"gx{mo}")
                with nc.allow_low_precision(reason="bf16 gelu"):
                    nc.vector.tensor_copy(x[:], pp[:])
                    u = gelp.tile([128, 512], BF16, tag="gtmp",
                                  name=f"gu{mo}")
                    nc.vector.tensor_mul(u[:], x[:], x[:])          # x^2
                    t1 = gelp.tile([128, 512], BF16, tag="gtmp",
                                   name=f"gt1{mo}")
                    # t1 = (x*GC*GC1)*u = GC*GC1*x^3
                    nc.vector.scalar_tensor_tensor(
                        t1[:], x[:], GC * GC1, u[:], ALU.mult, ALU.mult)
                    w = gelp.tile([128, 512], BF16, tag="gtmp",
                                  name=f"gw{mo}")
                    # w = (x*GC) + t1
                    nc.vector.scalar_tensor_tensor(
                        w[:], x[:], GC, t1[:], ALU.mult, ALU.add)
                    t = gelp.tile([128, 512], BF16, tag="gtmp",
                                  name=f"gt{mo}")
                    nc.scalar.activation(t[:], w[:], AF.Tanh)
                    # t2 = t*0.5 + 0.5
                    t2 = gelp.tile([128, 512], BF16, tag="gtmp",
                                   name=f"gt2{mo}")
                    nc.vector.tensor_scalar(t2[:], t[:], 0.5, 0.5,
                                            ALU.mult, ALU.add)
                    # out = (x*1.0) * t2 -> l2v slice, laid out (c, mo, d)
                    nc.vector.scalar_tensor_tensor(
                        l2v[:].rearrange("p (c m d) -> p c m d", c=HEADS,
                                         m=MT2)[:, :, mo, :],
                        x[:], 1.0, t2[:], ALU.mult, ALU.mult)

            def send_v_all():
                # keep (m d) fused: 512B-contiguous runs, 1024 descriptors
                nc.sync.dma_start(
                    send_v[:].rearrange("c p m d -> p c (m d)"),
                    l2v[:].rearrange("p (c md) -> p c md", c=HEADS))
                nc.gpsimd.collective_compute(
                    "AllToAll", mybir.AluOpType.bypass,
                    replica_groups=[list(range(N_CORES))],
                    ins=[send_v.opt()], outs=[recv_v.opt()])
                # recv side: vaug loads (m = b*2+h row-chunk)
                for b in range(N):
                    for h in range(2):
                        nc.sync.dma_start(
                            vaug[:].rearrange("p (j b h e) -> p j b h e",
                                              j=HEADS, b=N, h=2)
                            [:, :, b, h, 0:64],
                            recv_v[:, :, b * 2 + h, :]
                            .rearrange("j p d -> p j d"))

            po = {}

            def pv(b, qc):
                nk = 4 * qc + 4
                p = psp.tile([128, 1024], F32, tag="ps", name=f"po{b}{qc}")
                po[(b, qc)] = p
                for kt in range(nk):
                    g, h = kt // 2, kt % 2
                    gidx = (kt // 2) * 4 + b * 2 + (kt % 2)
                    nc.tensor.matmul(
                        p[0:65, 0:512],
                        vaug[:, gidx * 65:(gidx + 1) * 65],
                        exps[(b, qc)][g][:, h * 512:(h + 1) * 512],
                        start=(kt == 0), stop=(kt == nk - 1))

            def norm(b, qc):
                p = po[(b, qc)]
                d_sb = smp.tile([1, 512], F32, tag="d", name=f"d{b}{qc}")
                nc.vector.tensor_copy(d_sb[:], p[64:65, 0:512])
                r = smp.tile([1, 512], F32, tag="r", name=f"r{b}{qc}")
                with nc.allow_low_precision(reason="approx recip"):
                    nc.vector.reciprocal_approx_fast(r[:], d_sb[:])
                rb = smp.tile([64, 512], F32, tag="rb", name=f"rb{b}{qc}")
                nc.gpsimd.partition_broadcast(rb[:], r[:], channels=64)
                with nc.allow_low_precision(reason="bf16"):
                    nc.vector.tensor_mul(
                        oT_all[:, b * S + qc * 512:b * S + (qc + 1) * 512],
                        p[0:64, 0:512], rb[:])
                nc.sync.dma_start(
                    aT_out[b, :, qc * 512:(qc + 1) * 512],
                    oT_all[:, b * S + qc * 512:b * S + (qc + 1) * 512])

            # ------- interleaved scores + v-L2 + sends (PE queue order) ----
            groups = [(qc, g) for qc in (3, 2, 1, 0)
                      for g in range(2 * (qc + 1))]   # 20 groups
            # PE interleave: 2 groups, then a v-L2 chunk, ...
            gi = 0

            def emit_groups(n):
                nonlocal gi
                for _ in range(n):
                    if gi < len(groups):
                        score_group(*groups[gi])
                        gi += 1

            emit_groups(2)
            v_l2_chunk(0)
            emit_groups(2)
            v_l2_chunk(1)
            emit_groups(2)
            v_l2_chunk(2)
            emit_groups(2)
            v_l2_chunk(3)
            send_v_all()
            # non-latency-critical output stores after the v send
            nc.sync.dma_start(
                kT_out.ap().rearrange("(m m2) p r -> (m2 p) m r", m2=2),
                l2k[:].rearrange("p (m r) -> p m r", m=MT2))
            nc.sync.dma_start(v_out.ap(), l2v[:])
            emit_groups(len(groups))

            # ---------------- PV + norms ----------------
            pv(0, 3)
            pv(1, 3)
            norm(0, 3)
            pv(0, 2)
            norm(1, 3)
            pv(1, 2)
            norm(0, 2)
            pv(0, 1)
            norm(1, 2)
            pv(1, 1)
            norm(0, 1)
            pv(0, 0)
            norm(1, 1)
            pv(1, 0)
            norm(0, 0)
            norm(1, 0)

    nc.compile()
    return nc


_COMPILED = None


def _get_compiled():
    global _COMPILED
    if _COMPILED is None:
        _COMPILED = _build()
    return _COMPILED


def _band_mask():
    return (np.arange(1024, dtype=np.int32)[None, :]
            >= (np.arange(128, dtype=np.int32)[:, None] + 512)).astype(
                np.float32)


def _bf16(a):
    return np.ascontiguousarray(np.asarray(a, dtype=np.float32)
                                .astype(ml_dtypes.bfloat16))


def _pack_w1(w):            # [512, 2048] -> [128, (m kt 128)]
    w = np.asarray(w, np.float32)
    return _bf16(w.reshape(KT1, 128, MT1, 128).transpose(1, 2, 0, 3)
                 .reshape(128, MT1 * KT1 * 128))


def _pack_w2(w):            # [2048, 512] -> [128, (kt d)]
    w = np.asarray(w, np.float32)
    return _bf16(w.reshape(MT1, 128, D).transpose(1, 0, 2)
                 .reshape(128, MT1 * D))


def _make_in_maps(x, qW1, qb1, qW2, qb2, kW1, kb1, kW2, kb2, vW1, vb1,
                  vW2, vb2):
    x = np.asarray(x, np.float32)
    cf32 = np.zeros((128, CF_W), np.float32)
    for t, b1 in (("k", kb1), ("q", qb1), ("v", vb1)):
        cf32[:, CF_B1[t]:CF_B1[t] + 16] = np.asarray(b1, np.float32) \
            .reshape(16, 128).T
    for t, b2 in (("k", kb2), ("q", qb2)):
        cf32[:, CF_B2[t]:CF_B2[t] + 4] = np.asarray(b2, np.float32) \
            .reshape(4, 128).T
    cbf = np.zeros((128, CB_W), np.float32)
    cbf[:, CB_BAND:CB_BAND + 1024] = _band_mask()
    cbf[0, CB_B2V:CB_B2V + 512] = np.asarray(vb2, np.float32)
    shared = {
        "w1q": _pack_w1(qW1), "w1k": _pack_w1(kW1), "w1v": _pack_w1(vW1),
        "w2q": _pack_w2(qW2), "w2k": _pack_w2(kW2), "w2v": _pack_w2(vW2),
        "cf32": cf32, "cbf": cbf.astype(ml_dtypes.bfloat16),
    }
    in_maps = []
    for c in range(N_CORES):
        xc = np.concatenate([x[b, c * RPC:(c + 1) * RPC, :]
                             for b in range(N)], 0)       # [ROWS, D]
        xT = np.ascontiguousarray(xc.T)                   # [D, ROWS]
        im = dict(shared)
        im["xt"] = _bf16(xT.reshape(KT1, 128, ROWS).transpose(1, 0, 2)
                         .reshape(128, KT1 * ROWS))
        in_maps.append(im)
    return in_maps


def _assemble(res):
    k_full = np.empty((N, S, D), np.float32)
    v_full = np.empty((N, S, D), np.float32)
    out_full = np.empty((N, S, D), np.float32)
    for j in range(N_CORES):
        kT_j = np.asarray(res[j]["kT_out"], np.float32)   # [8, 64, ROWS]
        v_j = np.asarray(res[j]["v_out"], np.float32) \
            .reshape(128, HEADS, N, 2, Dh).transpose(2, 3, 0, 1, 4) \
            .reshape(N, RPC, D)                           # [N, RPC, D]
        aT_j = np.asarray(res[j]["attn_outT"], np.float32)  # [N, Dh, S]
        kk = kT_j.reshape(HEADS, Dh, N, RPC).transpose(2, 3, 0, 1) \
            .reshape(N, RPC, D)
        for b in range(N):
            k_full[b, j * RPC:(j + 1) * RPC, :] = kk[b]
            v_full[b, j * RPC:(j + 1) * RPC, :] = v_j[b]
            out_full[b, :, j * Dh:(j + 1) * Dh] = aT_j[b].T
    return k_full, v_full, out_full


def kernel(**inputs):
    nc = _get_compiled()
    in_maps = _make_in_maps(**inputs)
    res = run_bass_kernel_spmd(nc, in_maps, list(range(N_CORES))).results
    return _assemble(res)
